# revision 30
# baseline (speedup 1.0000x reference)
"""Trainium2 Bass kernel for nn_Attention: y = softmax((xW_q)(xW_k)^T/sqrt(d)) (xW_v).

Full inputs: x [16, 512, 4, 256] f32, W_qkv [768, 256] f32 (torch Linear layout).
The reference flattens (n, h) -> 2048 tokens and splits the 768 projection
outputs interleaved (stride 3) into q/k/v of width 256 each; attention runs
over the flat 2048-token axis with head dim 256.

Sharding: data-parallel over batch, 2 batches per core on 8 cores. W replicated.

Key algebraic move: S = (xWq^T)(xWk^T)^T = x M x^T with M = Wq^T Wk folded on
the host, so ONE device projection y = xM replaces the q and k projections.

All PE-facing tensors are bf16 (host pre-rounds x^T / M / Wv^T, halving the
input DMA bytes; bf16 LDWEIGHTS also hides fully under every matmul, where
fp32 weight loads were the throughput limiter). PSUM accumulation is fp32.

Per-core device graph (2048-token, d=256 attention per batch):
  - x^T [256, 2048] bf16 staged in SBUF per batch (quartered DMAs so the
    first matmul group starts after ~256 KB).
  - y^T = M-stationary matmuls -> f32 PSUM -> bf16 SBUF.
  - S^T[j, i] = x-stationary matmuls (moving y^T) -> PSUM; ScalarE exp over
    2-bank PSUM spans (scale fused; no max subtraction: |S*scale| <~ 6 for
    N(0,1) inputs) writes P^T to SBUF in bf16, already transposed for the
    P@V contraction over j.
  - v = x-stationary matmuls (moving Wv^T), stored [j, o] with a ones column
    so P@V also accumulates the softmax row-sum, plus a pad column.
  - P@V: P^T-stationary matmuls accumulate over the 16 j-chunks into
    [i-chunk, 258] f32 PSUM; epilogue multiplies by the reciprocal of the
    ones-column on VectorE and DMAs out.
  - Software pipelining: every engine-idle window on the PE is filled by
    interleaving independent matmul groups (v-projection under S^T of slice
    0, P@V of slice s under S^T of slice s+1, the last slice's own P@V under
    its S^T tail), and ~6us of throwaway warm-up matmuls run during the
    initial DMA wait so the HAM clock gate reaches 2.4 GHz before real work.
Output [2, 2048, 256] per core; host concatenates and reshapes.
"""

import sys

for _p in ("/opt/trn_rl_repo",):
    if _p not in sys.path:
        sys.path.insert(0, _p)

import numpy as np

B, N, H, D = 16, 512, 4, 256
SEQ = N * H          # 2048 flat tokens
NCORES = 8
BPC = B // NCORES    # batches per core
SCALE = float(D) ** -0.5

_CACHE = {}


def _build_nc():
    import concourse.mybir as mybir
    import concourse.tile as tile
    from concourse import bacc

    f32 = mybir.dt.float32
    bf16 = mybir.dt.bfloat16
    EXP = mybir.ActivationFunctionType.Exp

    nc = bacc.Bacc("TRN2", target_bir_lowering=False, debug=False)
    xT_ext = nc.declare_dram_parameter("xT", [BPC, D, SEQ], bf16, isOutput=False)
    wm_ext = nc.declare_dram_parameter("wm", [D, D], bf16, isOutput=False)
    wv_ext = nc.declare_dram_parameter("wv", [D, D], bf16, isOutput=False)
    out_ext = nc.declare_dram_parameter("out", [BPC, SEQ, D], f32, isOutput=True)

    DC = D // 128        # 2 contraction chunks of the 256-dim
    NJ = SEQ // 128      # 16 j-chunks
    NI = SEQ // 512      # 4 i-slices of 512
    VW = D + 2           # 258: v plus ones column plus pad

    with tile.TileContext(nc) as tc:
        with (
            tc.tile_pool(name="consts", bufs=1) as consts,
            tc.tile_pool(name="xt", bufs=2) as xt_pool,
            tc.tile_pool(name="qkv", bufs=2) as qkv_pool,
            tc.tile_pool(name="pt", bufs=20) as pt_pool,
            tc.tile_pool(name="eout", bufs=4) as eout_pool,
            tc.tile_pool(name="ps2", bufs=2, space="PSUM") as ps2,
            tc.tile_pool(name="ps1", bufs=4, space="PSUM") as ps1,
        ):
            # PE warm-up (see module docstring).
            warm_w = consts.tile([128, 128], bf16, tag="warm_w")
            nc.gpsimd.memset(warm_w[:], 0.0)
            warm_x = consts.tile([128, 512], bf16, tag="warm_x")
            nc.gpsimd.memset(warm_x[:], 0.0)
            warm_ps = ps1.tile([128, 512], f32, tag="ps1")
            for _ in range(9):
                nc.tensor.matmul(warm_ps[:], warm_w[:], warm_x[:], start=True, stop=True)

            # wm first (head of the DMA queue: first matmul group's dep);
            # wv follows the first x^T half via load_consts_rest().
            wm_sb = consts.tile([128, DC, D], bf16, tag="wm")
            for ac in range(DC):
                nc.sync.dma_start(
                    out=wm_sb[:, ac, :], in_=wm_ext[ac * 128 : (ac + 1) * 128, :]
                )
            wv_bf = consts.tile([128, DC, D], bf16, tag="wv")
            ones_sb = consts.tile([128, 1], f32, tag="ones")
            nc.vector.memset(ones_sb[:], 1.0)

            def load_consts_rest():
                for ac in range(DC):
                    nc.sync.dma_start(
                        out=wv_bf[:, ac, :], in_=wv_ext[ac * 128 : (ac + 1) * 128, :]
                    )

            # Deferred P@V: emit_pav(src, g) emits one half (8 of 16 j-steps)
            # of one i-chunk's accumulation for slice `src`, so P@V matmuls
            # slot between other matmul groups on the PE.
            op_live = {}

            def emit_pav(src, g):
                if src is None:
                    return
                bb, isl, pts, v_prev = src
                ic = g // 2
                half = g % 2
                key = (bb, isl, ic)
                if half == 0:
                    op_tile = ps1.tile([128, VW], f32, tag="ps1")
                    op_live[key] = op_tile
                op = op_live[key]
                for jc in range(half * 8, half * 8 + 8):
                    nc.tensor.matmul(
                        op[:],
                        pts[jc // 2][:, jc % 2, ic * 128 : (ic + 1) * 128],
                        v_prev[:, jc, :],
                        start=(jc == 0),
                        stop=(jc == NJ - 1),
                    )
                if half == 1:
                    rec = eout_pool.tile([128, 1], f32, tag="rec")
                    nc.vector.reciprocal(rec[:], op[:, D : D + 1])
                    osb = eout_pool.tile([128, D], f32, tag="osb")
                    nc.vector.tensor_scalar_mul(osb[:], op[:, 0:D], rec[:])
                    i0 = isl * 512 + ic * 128
                    nc.sync.dma_start(out=out_ext[bb, i0 : i0 + 128, :], in_=osb[:])
                    del op_live[key]

            prev = None
            pending_h1 = []
            for bb in range(BPC):
                # Quartered loads so the first groups start after ~320 KB.
                xt_bf = xt_pool.tile([128, DC, SEQ], bf16, tag="xtb")
                for ih in range(2):
                    for ac in range(DC):
                        nc.sync.dma_start(
                            out=xt_bf[:, ac, ih * 1024 : (ih + 1) * 1024],
                            in_=xT_ext[
                                bb, ac * 128 : (ac + 1) * 128, ih * 1024 : (ih + 1) * 1024
                            ],
                        )
                    if bb == 0 and ih == 0:
                        load_consts_rest()

                yT = qkv_pool.tile([128, DC, SEQ], bf16, tag="yT")
                v_sb = qkv_pool.tile([128, NJ, VW], bf16, tag="v")
                nc.vector.tensor_copy(
                    v_sb[:, :, D:VW], ones_sb[:].to_broadcast([128, NJ, VW - D])
                )

                def emit_vproj(jc):
                    ps = ps1.tile([128, D], f32, tag="ps1")
                    for ac in range(DC):
                        nc.tensor.matmul(
                            ps[:],
                            xt_bf[:, ac, jc * 128 : (jc + 1) * 128],
                            wv_bf[:, ac, :],
                            start=(ac == 0),
                            stop=(ac == DC - 1),
                        )
                    nc.vector.tensor_copy(v_sb[:, jc, 0:D], ps[:])

                # Projection phase. The v-projections (all ih0-dependent)
                # are front-loaded so the ih1 y-groups, which wait on the
                # second x^T half's DMA, sit late in the PE FIFO; the
                # previous batch's final P@V half-1 chunks also weave in.
                def emit_yproj(ih, bc):
                    ps = ps2.tile([128, 2, 512], f32, tag="ps2")
                    for half in range(2):
                        isl = ih * 2 + half
                        for ac in range(DC):
                            nc.tensor.matmul(
                                ps[:, half, :],
                                wm_sb[:, ac, bc * 128 : (bc + 1) * 128],
                                xt_bf[:, ac, isl * 512 : (isl + 1) * 512],
                                start=(ac == 0),
                                stop=(ac == DC - 1),
                            )
                    nc.vector.tensor_copy(
                        yT[:, bc, ih * 1024 : (ih + 1) * 1024],
                        ps[:].rearrange("p a b -> p (a b)"),
                    )

                sched = [
                    ("y", 0, 0), ("v", 0), ("v", 1),
                    ("y", 0, 1), ("v", 2), ("v", 3), ("v", 4), ("v", 5),
                    ("y", 1, 0), ("v", 6), ("v", 7),
                    ("y", 1, 1),
                ]
                for item in sched:
                    if item[0] == "y":
                        emit_yproj(item[1], item[2])
                        if pending_h1:
                            emit_pav(*pending_h1.pop(0))
                    else:
                        emit_vproj(item[1])

                for isl in range(NI):
                    last_of_batch = isl == NI - 1
                    pts = []
                    cur = (bb, isl, pts, v_sb)
                    for g in range(8):  # two j-chunks per S^T group
                        sp = ps2.tile([128, 2, 512], f32, tag="ps2")
                        for half in range(2):
                            jc = g * 2 + half
                            for bc in range(DC):
                                nc.tensor.matmul(
                                    sp[:, half, :],
                                    xt_bf[:, bc, jc * 128 : (jc + 1) * 128],
                                    yT[:, bc, isl * 512 : (isl + 1) * 512],
                                    start=(bc == 0),
                                    stop=(bc == DC - 1),
                                )
                        pt = pt_pool.tile([128, 2, 512], bf16)
                        nc.scalar.activation(pt[:], sp[:], EXP, scale=SCALE)
                        pts.append(pt)
                        if isl == 0:
                            emit_vproj(8 + g)
                        else:
                            emit_pav(prev, g)
                        if last_of_batch and g >= 4:
                            emit_pav(cur, (g - 4) * 2)  # own half-0 chunks
                    if last_of_batch:
                        # Half-1 chunks ride inside the next batch's projection
                        # phase; for the final batch there is none, so flush.
                        pending_h1 = [(cur, ic * 2 + 1) for ic in range(4)]
                        prev = None
                    else:
                        prev = cur

            for args in pending_h1:
                emit_pav(*args)

    nc.compile()
    return nc


def _get_nc():
    if "nc" not in _CACHE:
        _CACHE["nc"] = _build_nc()
    return _CACHE["nc"]


def _prep_in_maps(x, W_qkv):
    import ml_dtypes

    bf = ml_dtypes.bfloat16
    x = np.ascontiguousarray(x, dtype=np.float32)
    W = np.ascontiguousarray(W_qkv, dtype=np.float32)
    xT = np.ascontiguousarray(
        x.reshape(B, SEQ, D).transpose(0, 2, 1).astype(bf)
    )
    wq = W[0::3, :]
    wk = W[1::3, :]
    wm = np.ascontiguousarray(
        (wq.T.astype(np.float64) @ wk.astype(np.float64)).astype(bf)
    )
    wvT = np.ascontiguousarray(W[2::3, :].T.astype(bf))
    return [
        {"xT": xT[c * BPC : (c + 1) * BPC], "wm": wm, "wv": wvT}
        for c in range(NCORES)
    ]


def _run(x, W_qkv, trace=False, tmpdir=None):
    from concourse.bass_utils import run_bass_kernel_spmd

    nc = _get_nc()
    in_maps = _prep_in_maps(x, W_qkv)
    res = run_bass_kernel_spmd(
        nc, in_maps, core_ids=list(range(NCORES)), trace=trace, tmpdir=tmpdir
    )
    out = np.concatenate([res.results[c]["out"] for c in range(NCORES)], axis=0)
    return out.reshape(B, N, H, D).astype(np.float32), res


def kernel(x, W_qkv):
    out, _ = _run(x, W_qkv)
    return out


# revision 31
# speedup vs baseline: 1.0027x; 1.0027x over previous
"""Trainium2 Bass kernel for nn_Attention: y = softmax((xW_q)(xW_k)^T/sqrt(d)) (xW_v).

Full inputs: x [16, 512, 4, 256] f32, W_qkv [768, 256] f32 (torch Linear layout).
The reference flattens (n, h) -> 2048 tokens and splits the 768 projection
outputs interleaved (stride 3) into q/k/v of width 256 each; attention runs
over the flat 2048-token axis with head dim 256.

Sharding: data-parallel over batch, 2 batches per core on 8 cores. W replicated.

Key algebraic move: S = (xWq^T)(xWk^T)^T = x M x^T with M = Wq^T Wk folded on
the host, so ONE device projection y = xM replaces the q and k projections.

All PE-facing tensors are bf16 (host pre-rounds x^T / M / Wv^T, halving the
input DMA bytes; bf16 LDWEIGHTS also hides fully under every matmul, where
fp32 weight loads were the throughput limiter). PSUM accumulation is fp32.

Per-core device graph (2048-token, d=256 attention per batch):
  - x^T [256, 2048] bf16 staged in SBUF per batch (quartered DMAs so the
    first matmul group starts after ~256 KB).
  - y^T = M-stationary matmuls -> f32 PSUM -> bf16 SBUF.
  - S^T[j, i] = x-stationary matmuls (moving y^T) -> PSUM; ScalarE exp over
    2-bank PSUM spans (scale fused; no max subtraction: |S*scale| <~ 6 for
    N(0,1) inputs) writes P^T to SBUF in bf16, already transposed for the
    P@V contraction over j.
  - v = x-stationary matmuls (moving Wv^T), stored [j, o] with a ones column
    so P@V also accumulates the softmax row-sum, plus a pad column.
  - P@V: P^T-stationary matmuls accumulate over the 16 j-chunks into
    [i-chunk, 258] f32 PSUM; epilogue multiplies by the reciprocal of the
    ones-column on VectorE and DMAs out.
  - Software pipelining: every engine-idle window on the PE is filled by
    interleaving independent matmul groups (v-projection under S^T of slice
    0, P@V of slice s under S^T of slice s+1, the last slice's own P@V under
    its S^T tail), and ~6us of throwaway warm-up matmuls run during the
    initial DMA wait so the HAM clock gate reaches 2.4 GHz before real work.
Output [2, 2048, 256] per core; host concatenates and reshapes.
"""

import sys

for _p in ("/opt/trn_rl_repo",):
    if _p not in sys.path:
        sys.path.insert(0, _p)

import numpy as np

B, N, H, D = 16, 512, 4, 256
SEQ = N * H          # 2048 flat tokens
NCORES = 8
BPC = B // NCORES    # batches per core
SCALE = float(D) ** -0.5

_CACHE = {}


def _build_nc():
    import concourse.mybir as mybir
    import concourse.tile as tile
    from concourse import bacc

    f32 = mybir.dt.float32
    bf16 = mybir.dt.bfloat16
    EXP = mybir.ActivationFunctionType.Exp

    nc = bacc.Bacc("TRN2", target_bir_lowering=False, debug=False)
    xT_ext = nc.declare_dram_parameter("xT", [BPC, D, SEQ], bf16, isOutput=False)
    wm_ext = nc.declare_dram_parameter("wm", [D, D], bf16, isOutput=False)
    wv_ext = nc.declare_dram_parameter("wv", [D, D], bf16, isOutput=False)
    out_ext = nc.declare_dram_parameter("out", [BPC, SEQ, D], f32, isOutput=True)

    DC = D // 128        # 2 contraction chunks of the 256-dim
    NJ = SEQ // 128      # 16 j-chunks
    NI = SEQ // 512      # 4 i-slices of 512
    VW = D + 2           # 258: v plus ones column plus pad

    with tile.TileContext(nc) as tc:
        with (
            tc.tile_pool(name="consts", bufs=1) as consts,
            tc.tile_pool(name="xt", bufs=2) as xt_pool,
            tc.tile_pool(name="qkv", bufs=2) as qkv_pool,
            tc.tile_pool(name="pt", bufs=20) as pt_pool,
            tc.tile_pool(name="eout", bufs=4) as eout_pool,
            tc.tile_pool(name="ps2", bufs=2, space="PSUM") as ps2,
            tc.tile_pool(name="ps1", bufs=4, space="PSUM") as ps1,
        ):
            # PE warm-up (see module docstring).
            warm_w = consts.tile([128, 128], bf16, tag="warm_w")
            nc.gpsimd.memset(warm_w[:], 0.0)
            warm_ps = ps1.tile([128, 128], f32, tag="ps1")
            for _ in range(32):
                nc.tensor.matmul(warm_ps[:], warm_w[:], warm_w[:], start=True, stop=True)

            # wm first (head of the DMA queue: first matmul group's dep);
            # wv follows the first x^T half via load_consts_rest().
            wm_sb = consts.tile([128, DC, D], bf16, tag="wm")
            for ac in range(DC):
                nc.sync.dma_start(
                    out=wm_sb[:, ac, :], in_=wm_ext[ac * 128 : (ac + 1) * 128, :]
                )
            wv_bf = consts.tile([128, DC, D], bf16, tag="wv")
            ones_sb = consts.tile([128, 1], f32, tag="ones")
            nc.vector.memset(ones_sb[:], 1.0)

            def load_consts_rest():
                for ac in range(DC):
                    nc.sync.dma_start(
                        out=wv_bf[:, ac, :], in_=wv_ext[ac * 128 : (ac + 1) * 128, :]
                    )

            # Deferred P@V: emit_pav(src, g) emits one half (8 of 16 j-steps)
            # of one i-chunk's accumulation for slice `src`, so P@V matmuls
            # slot between other matmul groups on the PE.
            op_live = {}

            def emit_pav(src, g):
                if src is None:
                    return
                bb, isl, pts, v_prev = src
                ic = g // 2
                half = g % 2
                key = (bb, isl, ic)
                if half == 0:
                    op_tile = ps1.tile([128, VW], f32, tag="ps1")
                    op_live[key] = op_tile
                op = op_live[key]
                for jc in range(half * 8, half * 8 + 8):
                    nc.tensor.matmul(
                        op[:],
                        pts[jc // 2][:, jc % 2, ic * 128 : (ic + 1) * 128],
                        v_prev[:, jc, :],
                        start=(jc == 0),
                        stop=(jc == NJ - 1),
                    )
                if half == 1:
                    rec = eout_pool.tile([128, 1], f32, tag="rec")
                    nc.vector.reciprocal(rec[:], op[:, D : D + 1])
                    osb = eout_pool.tile([128, D], f32, tag="osb")
                    nc.vector.tensor_scalar_mul(osb[:], op[:, 0:D], rec[:])
                    i0 = isl * 512 + ic * 128
                    nc.sync.dma_start(out=out_ext[bb, i0 : i0 + 128, :], in_=osb[:])
                    del op_live[key]

            prev = None
            pending_h1 = []
            for bb in range(BPC):
                # Quartered loads so the first groups start after ~320 KB.
                xt_bf = xt_pool.tile([128, DC, SEQ], bf16, tag="xtb")
                for ih in range(2):
                    for ac in range(DC):
                        nc.sync.dma_start(
                            out=xt_bf[:, ac, ih * 1024 : (ih + 1) * 1024],
                            in_=xT_ext[
                                bb, ac * 128 : (ac + 1) * 128, ih * 1024 : (ih + 1) * 1024
                            ],
                        )
                    if bb == 0 and ih == 0:
                        load_consts_rest()

                yT = qkv_pool.tile([128, DC, SEQ], bf16, tag="yT")
                v_sb = qkv_pool.tile([128, NJ, VW], bf16, tag="v")
                nc.vector.tensor_copy(
                    v_sb[:, :, D:VW], ones_sb[:].to_broadcast([128, NJ, VW - D])
                )

                def emit_vproj(jc):
                    ps = ps1.tile([128, D], f32, tag="ps1")
                    for ac in range(DC):
                        nc.tensor.matmul(
                            ps[:],
                            xt_bf[:, ac, jc * 128 : (jc + 1) * 128],
                            wv_bf[:, ac, :],
                            start=(ac == 0),
                            stop=(ac == DC - 1),
                        )
                    nc.vector.tensor_copy(v_sb[:, jc, 0:D], ps[:])

                # Projection phase. The v-projections (all ih0-dependent)
                # are front-loaded so the ih1 y-groups, which wait on the
                # second x^T half's DMA, sit late in the PE FIFO; the
                # previous batch's final P@V half-1 chunks also weave in.
                def emit_yproj(ih, bc):
                    ps = ps2.tile([128, 2, 512], f32, tag="ps2")
                    for half in range(2):
                        isl = ih * 2 + half
                        for ac in range(DC):
                            nc.tensor.matmul(
                                ps[:, half, :],
                                wm_sb[:, ac, bc * 128 : (bc + 1) * 128],
                                xt_bf[:, ac, isl * 512 : (isl + 1) * 512],
                                start=(ac == 0),
                                stop=(ac == DC - 1),
                            )
                    nc.vector.tensor_copy(
                        yT[:, bc, ih * 1024 : (ih + 1) * 1024],
                        ps[:].rearrange("p a b -> p (a b)"),
                    )

                sched = [
                    ("y", 0, 0), ("v", 0), ("v", 1),
                    ("y", 0, 1), ("v", 2), ("v", 3), ("v", 4), ("v", 5),
                    ("y", 1, 0), ("v", 6), ("v", 7),
                    ("y", 1, 1),
                ]
                for item in sched:
                    if item[0] == "y":
                        emit_yproj(item[1], item[2])
                        if pending_h1:
                            emit_pav(*pending_h1.pop(0))
                    else:
                        emit_vproj(item[1])

                for isl in range(NI):
                    last_of_batch = isl == NI - 1
                    pts = []
                    cur = (bb, isl, pts, v_sb)
                    for g in range(8):  # two j-chunks per S^T group
                        sp = ps2.tile([128, 2, 512], f32, tag="ps2")
                        for half in range(2):
                            jc = g * 2 + half
                            for bc in range(DC):
                                nc.tensor.matmul(
                                    sp[:, half, :],
                                    xt_bf[:, bc, jc * 128 : (jc + 1) * 128],
                                    yT[:, bc, isl * 512 : (isl + 1) * 512],
                                    start=(bc == 0),
                                    stop=(bc == DC - 1),
                                )
                        pt = pt_pool.tile([128, 2, 512], bf16)
                        nc.scalar.activation(pt[:], sp[:], EXP, scale=SCALE)
                        pts.append(pt)
                        if isl == 0:
                            emit_vproj(8 + g)
                        else:
                            emit_pav(prev, g)
                        if last_of_batch and g >= 4:
                            emit_pav(cur, (g - 4) * 2)  # own half-0 chunks
                    if last_of_batch:
                        # Half-1 chunks ride inside the next batch's projection
                        # phase; for the final batch there is none, so flush.
                        pending_h1 = [(cur, ic * 2 + 1) for ic in range(4)]
                        prev = None
                    else:
                        prev = cur

            for args in pending_h1:
                emit_pav(*args)

    nc.compile()
    return nc


def _get_nc():
    if "nc" not in _CACHE:
        _CACHE["nc"] = _build_nc()
    return _CACHE["nc"]


def _prep_in_maps(x, W_qkv):
    import ml_dtypes

    bf = ml_dtypes.bfloat16
    x = np.ascontiguousarray(x, dtype=np.float32)
    W = np.ascontiguousarray(W_qkv, dtype=np.float32)
    xT = np.ascontiguousarray(
        x.reshape(B, SEQ, D).transpose(0, 2, 1).astype(bf)
    )
    wq = W[0::3, :]
    wk = W[1::3, :]
    wm = np.ascontiguousarray(
        (wq.T.astype(np.float64) @ wk.astype(np.float64)).astype(bf)
    )
    wvT = np.ascontiguousarray(W[2::3, :].T.astype(bf))
    return [
        {"xT": xT[c * BPC : (c + 1) * BPC], "wm": wm, "wv": wvT}
        for c in range(NCORES)
    ]


def _run(x, W_qkv, trace=False, tmpdir=None):
    from concourse.bass_utils import run_bass_kernel_spmd

    nc = _get_nc()
    in_maps = _prep_in_maps(x, W_qkv)
    res = run_bass_kernel_spmd(
        nc, in_maps, core_ids=list(range(NCORES)), trace=trace, tmpdir=tmpdir
    )
    out = np.concatenate([res.results[c]["out"] for c in range(NCORES)], axis=0)
    return out.reshape(B, N, H, D).astype(np.float32), res


def kernel(x, W_qkv):
    out, _ = _run(x, W_qkv)
    return out
